# revision 1
# baseline (speedup 1.0000x reference)
"""Causal self-attention (separate heads) TRN2 Bass kernel.

Problem (hardcoded): B=4, T=2048, C=1024, H=16, HS=64, fp32.
  q/k/v = per-head linear projections of x; att = softmax(causal(q k^T / 8));
  y = att v; out = concat_heads(y) @ Wp.T + bp.

Sharding over 8 NeuronCores: core c -> batch b = c//2, head-group hg = c%2
(8 heads each). Each core computes a [T, C] partial of the output (its heads'
contribution through the column slice of Wp); host sums the two partials per
batch and adds bp.

All matmul operands are fp32r (full-rate PE, ~2e-4 matmul error).

Per-core schedule (single interleaved PE stream, ordered by block-granular
add_dep_helper chaining; attention is ACT-exp-throughput-bound, so the next
pair's Q/K projection matmuls are interleaved into the attention stream as PE
filler):
  init:     consts, bvrep broadcast (K=1 matmuls)
  V phase:  V[s, dm] for all 4 head pairs (xT streamed, contracted in PSUM
            over 4 c-chunks x 2 super-chunks, SBUF-side accumulate)
  QK(p0):   Q^T/K^T [dm, t] for pair 0 (same super-chunk scheme)
  stages p=0..3:  attention for pair p [S^T = K Q^T row-pair matmuls ->
            exp on ACT (scale=1/8, max-free softmax; S bounded ~20) ->
            causal via matmul column restriction + triangular mask multiply ->
            AV accumulation with ones-column-augmented V giving the softmax
            denominator in row 64 -> reciprocal + K=1 broadcast matmul
            normalization], interleaved with QK(p+1).
  C phase:  out[t, e] = sum_p YT_p^T @ WpT_p -> DMA out as [T, C].
"""
import numpy as np

from concourse import bacc, bass_utils, tile, mybir

B, T, C, H, HS = 4, 2048, 1024, 16, 64
NCORE = 8
NPAIR = 4
NCH = T // 512
NST = T // 128

f32 = mybir.dt.float32
f32r = mybir.dt.float32r
EXP = mybir.ActivationFunctionType.Exp

_CACHE = {}


def _build():
    nc = bacc.Bacc(None, target_bir_lowering=False)

    xT = nc.declare_dram_parameter("xT", [C, T], f32r, isOutput=False)
    wq = nc.declare_dram_parameter("wq", [128, 8, 512], f32r, isOutput=False)
    wk = nc.declare_dram_parameter("wk", [128, 8, 512], f32r, isOutput=False)
    wv = nc.declare_dram_parameter("wv", [128, 8, 512], f32r, isOutput=False)
    wp = nc.declare_dram_parameter("wp", [128, 4, 1024], f32r, isOutput=False)
    bq = nc.declare_dram_parameter("bq", [128, 4], f32, isOutput=False)
    bk = nc.declare_dram_parameter("bk", [128, 4], f32, isOutput=False)
    bv = nc.declare_dram_parameter("bv", [1, 512], f32r, isOutput=False)
    ones = nc.declare_dram_parameter("ones", [1, 128], f32r, isOutput=False)
    trimask = nc.declare_dram_parameter("trimask", [128, 128], f32r, isOutput=False)
    out = nc.declare_dram_parameter("out", [T, C], f32, isOutput=True)

    with tile.TileContext(nc) as tc:
        with tc.tile_pool(name="persist", bufs=1) as pp:
            # ---- constants / persistent tensors ----
            ones_sb = pp.tile([1, 128], f32r, tag="ones")
            tri_sb = pp.tile([128, 128], f32r, tag="tri")
            bq_sb = pp.tile([128, 4], f32, tag="bq")
            bk_sb = pp.tile([128, 4], f32, tag="bk")
            bv_sb = pp.tile([1, 512], f32r, tag="bv")
            nc.sync.dma_start(ones_sb[:], ones[:])
            nc.sync.dma_start(tri_sb[:], trimask[:])
            nc.sync.dma_start(bq_sb[:], bq[:])
            nc.sync.dma_start(bk_sb[:], bk[:])
            nc.sync.dma_start(bv_sb[:], bv[:])

            V = pp.tile([128, NPAIR, NST, 130], f32r, tag="V")

            onescol = pp.tile([128, 32], f32, tag="onescol")
            nc.vector.memset(onescol[:], 1.0)
            for p in range(NPAIR):
                for i in range(NST):
                    nc.vector.tensor_copy(V[:, p, i, 64:130:65], onescol[:, 0:2])

            bvrep = pp.tile([128, 512], f32, tag="bvrep")
            with tc.tile_pool(name="ps_init", bufs=2, space="PSUM") as ps_init:
                for p in range(NPAIR):
                    psb = ps_init.tile([128, 512], f32, tag="psb", name="psb")
                    nc.tensor.matmul(
                        psb[:, 0:128], ones_sb[:], bv_sb[:, 128 * p : 128 * p + 128],
                        start=True, stop=True,
                    )
                    nc.vector.tensor_copy(
                        bvrep[:, 128 * p : 128 * p + 128], psb[:, 0:128]
                    )

            # PE emission-order chain, block granular
            _chain = {"prev": None, "first": None}

            def pe_mm(*args, **kw):
                inst = nc.tensor.matmul(*args, **kw)
                if _chain["first"] is None and _chain["prev"] is not None:
                    tile.add_dep_helper(
                        inst.ins, _chain["prev"].ins, sync=False,
                        reason="pe block order",
                    )
                if _chain["first"] is None:
                    _chain["first"] = inst
                _chain["prev"] = inst
                return inst

            def end_blk():
                _chain["first"] = None

            with tc.tile_pool(name="phBC", bufs=1) as pb:
              YT = pb.tile([128, NPAIR, T], f32r, tag="YT")
              with (
                  tc.tile_pool(name="xcpool", bufs=5) as pxc,
                  tc.tile_pool(name="wqk", bufs=2) as pwqk,
                  tc.tile_pool(name="qkt", bufs=2) as pqkt,
              ):
                qt_of = {}
                kt_of = {}

                def alloc_qkt(p):
                    qt_of[p] = pqkt.tile([128, T], f32r, tag="QTp", name="QTp")
                    kt_of[p] = pqkt.tile([128, T], f32r, tag="KTp", name="KTp")
                # ================= V phase (all pairs) =================
                with (
                    tc.tile_pool(name="wvpool", bufs=1) as pwv,
                    tc.tile_pool(name="ps_V", bufs=6, space="PSUM") as psv,
                ):
                    alloc_qkt(0)
                    wq0 = pwqk.tile([128, 8, 128], f32r, tag="wq_sl", name="wq_sl")
                    wk0 = pwqk.tile([128, 8, 128], f32r, tag="wk_sl", name="wk_sl")
                    nc.sync.dma_start(wq0[:], wq[:, :, 0:128])
                    nc.sync.dma_start(wk0[:], wk[:, :, 0:128])
                    for sc in range(2):
                        wv_sc = pwv.tile([128, 4, 512], f32r, tag="wv_sc")
                        nc.sync.dma_start(wv_sc[:], wv[:, 4 * sc : 4 * sc + 4, :])
                        xc = []
                        for kk in range(4):
                            k_abs = 4 * sc + kk
                            xt = pxc.tile([128, T], f32r, tag="xc", name="xc")
                            nc.sync.dma_start(
                                xt[:], xT[128 * k_abs : 128 * k_abs + 128, :]
                            )
                            xc.append(xt)
                        for st in range(NST):
                            ps = psv.tile([128, 512], f32, tag="ps_v", name="ps_v")
                            for kk in range(4):
                                pe_mm(
                                    ps[:],
                                    xc[kk][:, 128 * st : 128 * st + 128],
                                    wv_sc[:, kk, :],
                                    start=(kk == 0),
                                    stop=(kk == 3),
                                )
                            for hh in range(2):
                                src = ps.rearrange(
                                    "s (p two d) -> s p two d", p=4, two=2
                                )[:, :, hh, :]
                                dst = V[:, :, st, 65 * hh : 65 * hh + 64]
                                if sc == 0:
                                    bsrc = bvrep.rearrange(
                                        "s (p two d) -> s p two d", p=4, two=2
                                    )[:, :, hh, :]
                                    nc.vector.tensor_add(dst, src, bsrc)
                                else:
                                    nc.vector.tensor_add(dst, src, dst)
                        end_blk()
                        # QK for pair 0 on the same xT stream
                        for w_sl, dest, bias_sb in (
                            (wq0, qt_of[0], bq_sb),
                            (wk0, kt_of[0], bk_sb),
                        ):
                            for tch in range(NCH):
                                ps = psv.tile([128, 512], f32, tag="ps_v", name="ps_qk0")
                                for kk in range(4):
                                    pe_mm(
                                        ps[:],
                                        w_sl[:, 4 * sc + kk, :],
                                        xc[kk][:, 512 * tch : 512 * tch + 512],
                                        start=(kk == 0),
                                        stop=(kk == 3),
                                    )
                                end_blk()
                                dslice = dest[:, 512 * tch : 512 * tch + 512]
                                if sc == 0:
                                    nc.vector.tensor_scalar_add(
                                        dslice, ps[:], bias_sb[:, 0:1]
                                    )
                                else:
                                    nc.vector.tensor_add(dslice, ps[:], dslice)

                # ============ attention stages with QK interleave ============
                with (
                    tc.tile_pool(name="phB_es", bufs=18) as pes,
                    tc.tile_pool(name="phB_rep", bufs=2) as prep,
                    tc.tile_pool(name="ps_work", bufs=4, space="PSUM") as pwork,
                    tc.tile_pool(name="ps_Y", bufs=2, space="PSUM") as psy,
                ):
                    # -- QK projection emission units for one pair --
                    def qk_units(p):
                        """List of closures; each emits one PE block (or DMA
                        group) of pair p's Q/K projection."""
                        state = {}

                        def load_wslices():
                            alloc_qkt(p)
                            wq_sl = pwqk.tile(
                                [128, 8, 128], f32r, tag="wq_sl", name="wq_sl"
                            )
                            wk_sl = pwqk.tile(
                                [128, 8, 128], f32r, tag="wk_sl", name="wk_sl"
                            )
                            nc.sync.dma_start(
                                wq_sl[:], wq[:, :, 128 * p : 128 * p + 128]
                            )
                            nc.sync.dma_start(
                                wk_sl[:], wk[:, :, 128 * p : 128 * p + 128]
                            )
                            state["w"] = {"q": wq_sl, "k": wk_sl}

                        def load_xc(sc):
                            def go():
                                xcs = []
                                for kk in range(4):
                                    k_abs = 4 * sc + kk
                                    xt = pxc.tile([128, T], f32r, tag="xc", name="xc")
                                    nc.sync.dma_start(
                                        xt[:],
                                        xT[128 * k_abs : 128 * k_abs + 128, :],
                                    )
                                    xcs.append(xt)
                                state[("xc", sc)] = xcs
                            return go

                        def combo(sc, proj, tch):
                            def go():
                                xcs = state[("xc", sc)]
                                w_sl = state["w"][proj]
                                dest = qt_of[p] if proj == "q" else kt_of[p]
                                bias_sb = bq_sb if proj == "q" else bk_sb
                                ps = pwork.tile([128, 512], f32, tag="pw", name="pw")
                                for kk in range(4):
                                    pe_mm(
                                        ps[:],
                                        w_sl[:, 4 * sc + kk, :],
                                        xcs[kk][:, 512 * tch : 512 * tch + 512],
                                        start=(kk == 0),
                                        stop=(kk == 3),
                                    )
                                end_blk()
                                dslice = dest[:, 512 * tch : 512 * tch + 512]
                                if sc == 0:
                                    nc.vector.tensor_scalar_add(
                                        dslice, ps[:], bias_sb[:, p : p + 1]
                                    )
                                else:
                                    nc.vector.tensor_add(dslice, ps[:], dslice)
                            return go

                        units = [load_wslices]
                        for sc in range(2):
                            units.append(load_xc(sc))
                            for proj in ("q", "k"):
                                for tch in range(NCH):
                                    units.append(combo(sc, proj, tch))
                        return units

                    def emit_tail(p, j, psY):
                        for hh in range(2):
                            row = prep.tile([1, 512], f32r, tag="row", name="row")
                            with nc.allow_low_precision(reason="f32r is 4-byte"):
                                nc.vector.reciprocal(row[:], psY[hh][64:65, :])
                            repc = prep.tile([64, 512], f32r, tag="repc", name="repc")
                            nc.gpsimd.partition_broadcast(repc[:], row[:])
                            nc.vector.tensor_mul(
                                YT[64 * hh : 64 * hh + 64, p, 512 * j : 512 * j + 512],
                                psY[hh][0:64, :],
                                repc[:],
                            )

                    GI = 2  # i-steps per attention block

                    for stage in range(NPAIR):
                        p = stage
                        filler = qk_units(p + 1) if p + 1 < NPAIR else []
                        fidx = 0

                        blocks = []  # (j, [(i, hh)...], last_of_chunk)
                        for j in range(NCH):
                            nst_j = 4 * j + 4
                            for i0 in range(0, nst_j, GI):
                                ii = list(range(i0, min(i0 + GI, nst_j)))
                                steps = [(i, hh) for i in ii for hh in range(2)]
                                blocks.append((j, steps, i0 + GI >= nst_j))

                        nfill = len(filler)
                        nblk = len(blocks)

                        eS_store = {}
                        psY_of = {}
                        tails_pending = []
                        for n in range(nblk + 3):
                            # deferred tails (release psY before next chunk's
                            # first AV allocates its slot)
                            while tails_pending and tails_pending[0][0] <= n:
                                _, tp, tj, tpsY = tails_pending.pop(0)
                                emit_tail(tp, tj, tpsY)
                            # AV block n-3 (deeper S->AV lookahead)
                            if n >= 3:
                                j, steps, last = blocks[n - 3]
                                psY = psY_of[j]
                                nst_j = 4 * j + 4
                                for (i, hh) in steps:
                                    off = max(0, 128 * i - 512 * j)
                                    eS = eS_store.pop((j, i, hh))
                                    pe_mm(
                                        psY[hh][:, off:512],
                                        V[:, p, i, 65 * hh : 65 * hh + 65],
                                        eS[:, off:512],
                                        start=(i == 0),
                                        stop=(i == nst_j - 1),
                                    )
                                end_blk()
                                if last:
                                    tails_pending.append((n + 1, p, j, psY))
                            # filler QK unit(s), front-loaded
                            want = min(nfill, ((n + 1) * nfill) // max(1, int(0.7 * nblk)))
                            while fidx < want:
                                filler[fidx]()
                                fidx += 1
                            # S block n
                            if n < nblk:
                                j, steps, last = blocks[n]
                                if j not in psY_of:
                                    psY_of[j] = [
                                        psy.tile(
                                            [65, 512], f32,
                                            tag=f"psY{hh}", name=f"psY{hh}",
                                        )
                                        for hh in range(2)
                                    ]
                                for (i, hh) in steps:
                                    off = max(0, 128 * i - 512 * j)
                                    h0 = 64 * hh
                                    psS = pwork.tile(
                                        [128, 512], f32, tag="pw", name="psS"
                                    )
                                    pe_mm(
                                        psS[:, off:512],
                                        kt_of[p][h0 : h0 + 64, 128 * i : 128 * i + 128],
                                        qt_of[p][
                                            h0 : h0 + 64,
                                            512 * j + off : 512 * j + 512,
                                        ],
                                        start=True,
                                        stop=True,
                                    )
                                    eS = pes.tile(
                                        [128, 512], f32r, tag="eS", name="eS"
                                    )
                                    nc.scalar.activation(
                                        eS[:, off:512], psS[:, off:512], EXP,
                                        scale=0.125,
                                    )
                                    if i >= 4 * j:
                                        nc.vector.tensor_mul(
                                            eS[:, off : off + 128],
                                            eS[:, off : off + 128],
                                            tri_sb[:],
                                        )
                                    eS_store[(j, i, hh)] = eS
                                end_blk()
                        while fidx < nfill:
                            filler[fidx]()
                            fidx += 1
                        while tails_pending:
                            _, tp, tj, tpsY = tails_pending.pop(0)
                            emit_tail(tp, tj, tpsY)

              # ================= C phase =================
              with (
                  tc.tile_pool(name="wppool", bufs=1) as pwp,
                  tc.tile_pool(name="phC", bufs=4) as pc,
                  tc.tile_pool(name="ps_O", bufs=2, space="PSUM") as pso,
              ):
                  wp_sb = pwp.tile([128, 4, 1024], f32r, tag="wp")
                  nc.sync.dma_start(wp_sb[:], wp[:])
                  for m in range(NST):
                      for e in range(2):
                          ps = pso.tile([128, 512], f32, tag="psO", name="psO")
                          for p in range(NPAIR):
                              pe_mm(
                                  ps[:],
                                  YT[:, p, 128 * m : 128 * m + 128],
                                  wp_sb[:, p, 512 * e : 512 * e + 512],
                                  start=(p == 0),
                                  stop=(p == 3),
                              )
                          end_blk()
                          ob = pc.tile([128, 512], f32, tag="ob", name="ob")
                          nc.vector.tensor_copy(ob[:], ps[:])
                          nc.sync.dma_start(
                              out[
                                  128 * m : 128 * m + 128,
                                  512 * e : 512 * e + 512,
                              ],
                              ob[:],
                          )

    nc.compile()
    return nc


def _prep_core_inputs(x, Wq, bq, Wk, bk, Wv, bv, Wp, core):
    b, hg = core // 2, core % 2
    h0 = 8 * hg

    def wprep(W):
        A = W[h0 : h0 + 8]
        Bm = np.transpose(A, (2, 0, 1)).reshape(C, 512)
        return np.ascontiguousarray(Bm.reshape(8, 128, 512).transpose(1, 0, 2))

    def bprep(bias):
        return np.ascontiguousarray(bias[h0 : h0 + 8].reshape(4, 128).T)

    wp_sl = Wp[:, 512 * hg : 512 * hg + 512]
    wp_prep = np.ascontiguousarray(wp_sl.T.reshape(4, 128, 1024).transpose(1, 0, 2))

    return {
        "xT": np.ascontiguousarray(x[b].T),
        "wq": wprep(Wq),
        "wk": wprep(Wk),
        "wv": wprep(Wv),
        "wp": wp_prep,
        "bq": bprep(bq),
        "bk": bprep(bk),
        "bv": np.ascontiguousarray(bv[h0 : h0 + 8].reshape(1, 512)),
        "ones": np.ones((1, 128), dtype=np.float32),
        "trimask": np.ascontiguousarray(np.triu(np.ones((128, 128), np.float32))),
    }


TRACE = False
TRACE_KW = {}


def kernel(x, Wq, bq, Wk, bk, Wv, bv, Wp, bp):
    x = np.asarray(x, np.float32)
    Wq = np.asarray(Wq, np.float32)
    bq = np.asarray(bq, np.float32)
    Wk = np.asarray(Wk, np.float32)
    bk = np.asarray(bk, np.float32)
    Wv = np.asarray(Wv, np.float32)
    bv = np.asarray(bv, np.float32)
    Wp = np.asarray(Wp, np.float32)
    bp = np.asarray(bp, np.float32)

    if "nc" not in _CACHE:
        _CACHE["nc"] = _build()
    nc = _CACHE["nc"]

    in_maps = [
        _prep_core_inputs(x, Wq, bq, Wk, bk, Wv, bv, Wp, core)
        for core in range(NCORE)
    ]
    res = bass_utils.run_bass_kernel_spmd(
        nc, in_maps, list(range(NCORE)), trace=TRACE, **TRACE_KW
    )
    _CACHE["last_result"] = res

    outp = np.empty((B, T, C), np.float32)
    for b in range(B):
        outp[b] = res.results[2 * b]["out"] + res.results[2 * b + 1]["out"] + bp
    return outp



# revision 10
# speedup vs baseline: 1.4658x; 1.4658x over previous
"""Causal self-attention (separate heads) TRN2 Bass kernel — v2 (bf16).

Problem (hardcoded): B=4, T=2048, C=1024, H=16, HS=64, fp32 in/out.
  q/k/v = per-head linear projections of x; att = softmax(causal(q k^T / 8));
  y = att v; out = concat_heads(y) @ Wp.T + bp.

Sharding over 8 NeuronCores: core c -> batch b = c//2, head-group hg = c%2
(8 heads each). Each core computes a [T, C] partial of the output (its heads'
contribution through the column slice of Wp); host sums the two partials per
batch and adds bp.

v2 changes vs v1 (461958 ns):
  - all matmul operands bf16 (fp32r measured ~0.85 ns/row on HW; bf16 streams
    1 col/cycle at the warm 2.4 GHz clock). PSUM accumulation stays fp32.
  - x and all weights DMA'd once at start and kept SBUF-resident (~90KB of
    208KB per partition) — no mid-kernel input DMA, no stage-boundary stalls.
  - softmax tail reciprocal via reciprocal_approx_fast (DVE InstReciprocal on
    a [1,512] row measured 3.3us each / 106us total in v1).
  - ~10 dummy warm-up matmuls at t=0 so the PE HAM clock-gate releases
    (1.2 -> 2.4 GHz) during the initial DMA wait.
  - QK projection filler spread by need-time (stage p's j-chunk needs qt/kt
    t-chunk <= j), and the c_proj (C phase) interleaved into stage 3 as its
    softmax tails complete (v1 ran stage 3 fully throttled with no filler).
  - out written bf16, host converts and sums partials.
"""
import numpy as np
import ml_dtypes

from concourse import bacc, bass_utils, tile, mybir

B, T, C, H, HS = 4, 2048, 1024, 16, 64
NCORE = 8
NPAIR = 4
NCH = T // 512
NST = T // 128

f32 = mybir.dt.float32
bf16 = mybir.dt.bfloat16
EXP = mybir.ActivationFunctionType.Exp

_CACHE = {}


def _build():
    nc = bacc.Bacc(None, target_bir_lowering=False)

    xall_d = nc.declare_dram_parameter("xall", [128, 8, T], bf16, isOutput=False)
    wq_d = nc.declare_dram_parameter("wq", [128, 8, 512], bf16, isOutput=False)
    wk_d = nc.declare_dram_parameter("wk", [128, 8, 512], bf16, isOutput=False)
    wv_d = nc.declare_dram_parameter("wv", [128, 8, 512], bf16, isOutput=False)
    wp_d = nc.declare_dram_parameter("wp", [128, 4, 1024], bf16, isOutput=False)
    bq_d = nc.declare_dram_parameter("bq", [128, 4], f32, isOutput=False)
    bk_d = nc.declare_dram_parameter("bk", [128, 4], f32, isOutput=False)
    bv_d = nc.declare_dram_parameter("bv", [1, 512], bf16, isOutput=False)
    tri_d = nc.declare_dram_parameter("trimask", [128, 128], bf16, isOutput=False)
    out = nc.declare_dram_parameter("out", [T, C], bf16, isOutput=True)

    with tile.TileContext(nc) as tc:
        with tc.tile_pool(name="persist", bufs=1) as pp:
            # ---- constants / persistent tensors ----
            ones_sb = pp.tile([1, 128], bf16, tag="ones")
            warm_rhs = pp.tile([1, 512], bf16, tag="warmrow")
            onescol = pp.tile([128, 32], f32, tag="onescol")
            nc.vector.memset(ones_sb[:], 1.0)
            nc.vector.memset(warm_rhs[:], 1.0)
            nc.vector.memset(onescol[:], 1.0)

            tri_sb = pp.tile([128, 128], bf16, tag="tri")
            bq_sb = pp.tile([128, 4], f32, tag="bq")
            bk_sb = pp.tile([128, 4], f32, tag="bk")
            bv_sb = pp.tile([1, 512], bf16, tag="bv")
            xall = pp.tile([128, 8, T], bf16, tag="xall")
            wv_sb = pp.tile([128, 8, 512], bf16, tag="wv")
            wq_sb = pp.tile([128, 8, 512], bf16, tag="wq")
            wk_sb = pp.tile([128, 8, 512], bf16, tag="wk")
            wp_sb = pp.tile([128, 4, 1024], bf16, tag="wp")

            # DMA priority order: tiny consts, then what the V phase needs
            # first (xall chunks 0-3 + wv), then the rest.
            nc.sync.dma_start(bv_sb[:], bv_d[:])
            nc.sync.dma_start(bq_sb[:], bq_d[:])
            nc.sync.dma_start(bk_sb[:], bk_d[:])
            nc.sync.dma_start(tri_sb[:], tri_d[:])
            for k in range(4):
                nc.sync.dma_start(xall[:, k, :], xall_d[:, k, :])
            nc.sync.dma_start(wv_sb[:], wv_d[:])
            for k in range(4, 8):
                nc.sync.dma_start(xall[:, k, :], xall_d[:, k, :])
            nc.sync.dma_start(wq_sb[:], wq_d[:])
            nc.sync.dma_start(wk_sb[:], wk_d[:])
            nc.sync.dma_start(wp_sb[:], wp_d[:])

            # per-(pair, i, hh) block padded 65 -> 72 cols (144B) so every
            # AV lhsT slice is 16B-aligned (bf16 FWL weight loads corrupt on
            # misaligned bases; fp32 had no FWL so v1's 130-wide layout worked)
            V = pp.tile([128, NPAIR, NST, 2, 72], bf16, tag="V")
            for p in range(NPAIR):
                for i in range(NST):
                    nc.vector.tensor_copy(V[:, p, i, :, 64], onescol[:, 0:2])

            # PE emission-order chain, block granular
            _chain = {"prev": None, "first": None}

            def pe_mm(*args, **kw):
                inst = nc.tensor.matmul(*args, **kw)
                if _chain["first"] is None and _chain["prev"] is not None:
                    tile.add_dep_helper(
                        inst.ins, _chain["prev"].ins, sync=False,
                        reason="pe block order",
                    )
                if _chain["first"] is None:
                    _chain["first"] = inst
                _chain["prev"] = inst
                return inst

            def end_blk():
                _chain["first"] = None

            bvrep = pp.tile([128, 512], f32, tag="bvrep")
            with tc.tile_pool(name="ps_init", bufs=2, space="PSUM") as ps_init:
                # HAM warm-up: ~4.3us of dummy matmuls (no DMA deps) so the
                # PE clock is at 2.4 GHz by the time real work starts.
                trash = ps_init.tile([128, 512], f32, tag="trash", name="trash")
                NWARM = 10
                for dnum in range(NWARM):
                    pe_mm(
                        trash[:], ones_sb[:], warm_rhs[:],
                        start=(dnum == 0), stop=(dnum == NWARM - 1),
                    )
                end_blk()
                for p in range(NPAIR):
                    psb = ps_init.tile([128, 512], f32, tag="psb", name="psb")
                    pe_mm(
                        psb[:, 0:128], ones_sb[:], bv_sb[:, 128 * p : 128 * p + 128],
                        start=True, stop=True,
                    )
                    end_blk()
                    nc.vector.tensor_copy(
                        bvrep[:, 128 * p : 128 * p + 128], psb[:, 0:128]
                    )

            with tc.tile_pool(name="phBC", bufs=1) as pb:
              YT = pb.tile([128, NPAIR, T], bf16, tag="YT")
              with tc.tile_pool(name="qkt", bufs=2) as pqkt:
                qt_of = {}
                kt_of = {}

                def alloc_qkt(p):
                    qt_of[p] = pqkt.tile([128, T], bf16, tag="QTp", name="QTp")
                    kt_of[p] = pqkt.tile([128, T], bf16, tag="KTp", name="KTp")

                def qk_unit_go(p, sc, proj, tch, pool, ptag):
                    if p not in qt_of:
                        alloc_qkt(p)
                    w_sb = wq_sb if proj == "q" else wk_sb
                    dest = qt_of[p] if proj == "q" else kt_of[p]
                    bias_sb = bq_sb if proj == "q" else bk_sb
                    ps = pool.tile([128, 512], f32, tag=ptag, name="ps_qk")
                    for kk in range(4):
                        pe_mm(
                            ps[:],
                            w_sb[:, 4 * sc + kk, 128 * p : 128 * p + 128],
                            xall[:, 4 * sc + kk, 512 * tch : 512 * tch + 512],
                            start=(kk == 0),
                            stop=(kk == 3),
                        )
                    end_blk()
                    dslice = dest[:, 512 * tch : 512 * tch + 512]
                    if sc == 0:
                        nc.vector.tensor_scalar_add(
                            dslice, ps[:], bias_sb[:, p : p + 1]
                        )
                    else:
                        nc.vector.tensor_add(dslice, ps[:], dslice)

                def qk_units(p, tchs, pool, ptag):
                    """One closure per (tch, proj, sc) PE block, tch-major so
                    early t-chunks complete first."""
                    units = []
                    for tch in tchs:
                        for proj in ("q", "k"):
                            for sc in range(2):
                                units.append(
                                    lambda p=p, sc=sc, proj=proj, tch=tch,
                                    pool=pool, ptag=ptag: qk_unit_go(
                                        p, sc, proj, tch, pool, ptag
                                    )
                                )
                    return units

                # ================= V phase (all pairs) + QK0 =================
                with tc.tile_pool(name="ps_V", bufs=6, space="PSUM") as psv:
                    for sc in range(2):
                        for st in range(NST):
                            ps = psv.tile([128, 512], f32, tag="ps_v", name="ps_v")
                            for kk in range(4):
                                pe_mm(
                                    ps[:],
                                    xall[:, 4 * sc + kk, 128 * st : 128 * st + 128],
                                    wv_sb[:, 4 * sc + kk, :],
                                    start=(kk == 0),
                                    stop=(kk == 3),
                                )
                            for hh in range(2):
                                src = ps.rearrange(
                                    "s (p two d) -> s p two d", p=4, two=2
                                )[:, :, hh, :]
                                dst = V[:, :, st, hh, 0:64]
                                if sc == 0:
                                    bsrc = bvrep.rearrange(
                                        "s (p two d) -> s p two d", p=4, two=2
                                    )[:, :, hh, :]
                                    nc.vector.tensor_add(dst, src, bsrc)
                                else:
                                    nc.vector.tensor_add(dst, src, dst)
                        end_blk()
                    # QK0 fully, after V (everything is SBUF-resident)
                    for u in qk_units(0, (0, 1, 2, 3), psv, "ps_v"):
                        u()

                # ============ attention stages with fillers ============
                with (
                    tc.tile_pool(name="phB_es", bufs=18) as pes,
                    tc.tile_pool(name="phB_rep", bufs=2) as prep,
                    tc.tile_pool(name="phC", bufs=4) as pc,
                    tc.tile_pool(name="ps_work", bufs=4, space="PSUM") as pwork,
                    tc.tile_pool(name="ps_Y", bufs=2, space="PSUM") as psy,
                ):
                    def emit_tail(p, j, psY):
                        for hh in range(2):
                            # reciprocal_approx_fast misreads PSUM sources
                            # (verified on HW) — stage the row through SBUF.
                            row = prep.tile([1, 512], f32, tag="row", name="row")
                            nc.vector.tensor_copy(row[:], psY[hh][64:65, :])
                            rrow = prep.tile([1, 512], f32, tag="rrow", name="rrow")
                            nc.vector.reciprocal_approx_fast(
                                out=rrow[:], in_=row[:]
                            )
                            repc = prep.tile([64, 512], f32, tag="repc", name="repc")
                            nc.gpsimd.partition_broadcast(repc[:], rrow[:])
                            nc.vector.tensor_mul(
                                YT[64 * hh : 64 * hh + 64, p, 512 * j : 512 * j + 512],
                                psY[hh][0:64, :],
                                repc[:],
                            )

                    def c_unit(m, e):
                        def go():
                            ps = pwork.tile([128, 512], f32, tag="pw", name="psC")
                            for pp_ in range(NPAIR):
                                pe_mm(
                                    ps[:],
                                    YT[:, pp_, 128 * m : 128 * m + 128],
                                    wp_sb[:, pp_, 512 * e : 512 * e + 512],
                                    start=(pp_ == 0),
                                    stop=(pp_ == 3),
                                )
                            end_blk()
                            ob = pc.tile([128, 512], bf16, tag="ob", name="ob")
                            nc.vector.tensor_copy(ob[:], ps[:])
                            nc.sync.dma_start(
                                out[
                                    128 * m : 128 * m + 128,
                                    512 * e : 512 * e + 512,
                                ],
                                ob[:],
                            )
                        return go

                    GI = 2  # i-steps per attention block

                    # filler assignment (stage -> QK units), by need-time:
                    # stage p's j-chunk needs qt/kt t-chunk <= j, so QK(p)
                    # t-chunks >= 1 may run inside stage p itself.
                    stage_fill = {
                        0: qk_units(1, (0, 1, 2), pwork, "pw"),
                        1: qk_units(1, (3,), pwork, "pw")
                        + qk_units(2, (0, 1), pwork, "pw"),
                        2: qk_units(2, (2, 3), pwork, "pw")
                        + qk_units(3, (0, 1), pwork, "pw"),
                        3: qk_units(3, (2, 3), pwork, "pw"),
                    }
                    nfill_est = {0: 12, 1: 12, 2: 16, 3: 8 + 32}

                    for stage in range(NPAIR):
                        p = stage
                        filler = stage_fill[p]
                        fidx = 0
                        last_stage = stage == NPAIR - 1

                        blocks = []  # (j, [(i, hh)...], last_of_chunk)
                        for j in range(NCH):
                            nst_j = 4 * j + 4
                            for i0 in range(0, nst_j, GI):
                                ii = list(range(i0, min(i0 + GI, nst_j)))
                                steps = [(i, hh) for i in ii for hh in range(2)]
                                blocks.append((j, steps, i0 + GI >= nst_j))

                        nblk = len(blocks)
                        nfe = nfill_est[p]

                        eS_store = {}
                        psY_of = {}
                        tails_pending = []

                        def pop_tails(n):
                            while tails_pending and tails_pending[0][0] <= n:
                                _, tp, tj, tpsY = tails_pending.pop(0)
                                emit_tail(tp, tj, tpsY)
                                if last_stage:
                                    for m in range(4 * tj, 4 * tj + 4):
                                        for e in range(2):
                                            filler.append(c_unit(m, e))

                        for n in range(nblk + 3):
                            pop_tails(n)
                            # AV block n-3 (deeper S->AV lookahead)
                            if n >= 3:
                                j, steps, last = blocks[n - 3]
                                psY = psY_of[j]
                                nst_j = 4 * j + 4
                                for (i, hh) in steps:
                                    off = max(0, 128 * i - 512 * j)
                                    eS = eS_store.pop((j, i, hh))
                                    pe_mm(
                                        psY[hh][:, off:512],
                                        V[:, p, i, hh, 0:65],
                                        eS[:, off:512],
                                        start=(i == 0),
                                        stop=(i == nst_j - 1),
                                    )
                                end_blk()
                                if last:
                                    tails_pending.append((n + 1, p, j, psY))
                            # filler unit(s), front-loaded, capped per block
                            want = min(
                                len(filler),
                                ((n + 1) * nfe) // max(1, int(0.7 * nblk)),
                                fidx + 6,
                            )
                            while fidx < want:
                                filler[fidx]()
                                fidx += 1
                            # S block n
                            if n < nblk:
                                j, steps, last = blocks[n]
                                if j not in psY_of:
                                    psY_of[j] = [
                                        psy.tile(
                                            [65, 512], f32,
                                            tag=f"psY{hh}", name=f"psY{hh}",
                                        )
                                        for hh in range(2)
                                    ]
                                for (i, hh) in steps:
                                    off = max(0, 128 * i - 512 * j)
                                    h0 = 64 * hh
                                    psS = pwork.tile(
                                        [128, 512], f32, tag="pw", name="psS"
                                    )
                                    pe_mm(
                                        psS[:, off:512],
                                        kt_of[p][h0 : h0 + 64, 128 * i : 128 * i + 128],
                                        qt_of[p][
                                            h0 : h0 + 64,
                                            512 * j + off : 512 * j + 512,
                                        ],
                                        start=True,
                                        stop=True,
                                    )
                                    eS = pes.tile(
                                        [128, 512], bf16, tag="eS", name="eS"
                                    )
                                    nc.scalar.activation(
                                        eS[:, off:512], psS[:, off:512], EXP,
                                        scale=0.125,
                                    )
                                    if i >= 4 * j:
                                        nc.vector.tensor_mul(
                                            eS[:, off : off + 128],
                                            eS[:, off : off + 128],
                                            tri_sb[:],
                                        )
                                    eS_store[(j, i, hh)] = eS
                                end_blk()
                        # drain: tails first (they append C units on stage 3)
                        pop_tails(10**9)
                        while fidx < len(filler):
                            filler[fidx]()
                            fidx += 1

    nc.compile()
    return nc


def _prep_core_inputs(x, Wq, bq, Wk, bk, Wv, bv, core):
    b, hg = core // 2, core % 2
    h0 = 8 * hg
    bft = ml_dtypes.bfloat16

    def wprep(W):
        A = W[h0 : h0 + 8]
        Bm = np.transpose(A, (2, 0, 1)).reshape(C, 512)
        return np.ascontiguousarray(
            Bm.reshape(8, 128, 512).transpose(1, 0, 2)
        ).astype(bft)

    def bprep(bias):
        return np.ascontiguousarray(bias[h0 : h0 + 8].reshape(4, 128).T)

    xT = x[b].T  # [C, T]
    xall = np.ascontiguousarray(
        xT.reshape(8, 128, T).transpose(1, 0, 2)
    ).astype(bft)

    return {
        "xall": xall,
        "wq": wprep(Wq),
        "wk": wprep(Wk),
        "wv": wprep(Wv),
        "bq": bprep(bq),
        "bk": bprep(bk),
        "bv": np.ascontiguousarray(bv[h0 : h0 + 8].reshape(1, 512)).astype(bft),
        "trimask": np.triu(np.ones((128, 128), np.float32)).astype(bft),
    }


def _prep_wp(Wp, hg):
    wp_sl = Wp[:, 512 * hg : 512 * hg + 512]
    return np.ascontiguousarray(
        wp_sl.T.reshape(4, 128, 1024).transpose(1, 0, 2)
    ).astype(ml_dtypes.bfloat16)


TRACE = False
TRACE_KW = {}


def kernel(x, Wq, bq, Wk, bk, Wv, bv, Wp, bp):
    x = np.asarray(x, np.float32)
    Wq = np.asarray(Wq, np.float32)
    bq = np.asarray(bq, np.float32)
    Wk = np.asarray(Wk, np.float32)
    bk = np.asarray(bk, np.float32)
    Wv = np.asarray(Wv, np.float32)
    bv = np.asarray(bv, np.float32)
    Wp = np.asarray(Wp, np.float32)
    bp = np.asarray(bp, np.float32)

    if "nc" not in _CACHE:
        _CACHE["nc"] = _build()
    nc = _CACHE["nc"]

    wp_of_hg = [_prep_wp(Wp, hg) for hg in range(2)]
    in_maps = []
    for core in range(NCORE):
        m = _prep_core_inputs(x, Wq, bq, Wk, bk, Wv, bv, core)
        m["wp"] = wp_of_hg[core % 2]
        in_maps.append(m)
    res = bass_utils.run_bass_kernel_spmd(
        nc, in_maps, list(range(NCORE)), trace=TRACE, **TRACE_KW
    )
    _CACHE["last_result"] = res

    outp = np.empty((B, T, C), np.float32)
    for b in range(B):
        outp[b] = (
            res.results[2 * b]["out"].astype(np.float32)
            + res.results[2 * b + 1]["out"].astype(np.float32)
            + bp
        )
    return outp
